# revision 4
# baseline (speedup 1.0000x reference)
"""GCN2Net Trainium2 kernel (8-core SPMD), v6.

v5 -> v6:
- Fitted cap profile: per-(tile, src-band) chunk caps derived from the actual
  graph (shared across cores), cutting gather padding ~15% -> ~4%.
- Unequal src bands/segs [31,31,28,10] tiles so the last AllGathers are small.
- fp8 AllGather: xnext is bulk-DMA-cast f16->fp8 before the collective and
  fp8->f16 after, halving collective wire time. Activations are pre-scaled by
  S=16 (network is positively homogeneous) to keep fp8 in normal range.
- Per-seg gather defers sized so AG+cast latency hides behind earlier segs.
- Batched DMAs: one xself read + one xnext write per 5-tile group, single
  3-dim DMA per init tile, one final out write.
"""
import math
import os
import numpy as np

P = 128
N_CORES = 8
N_NODES = 100000
IN_DIM = 512
HID = 128
N_LAYERS = 8
ALPHA = 0.1
THETA = 0.5
S_SCALE = 16.0

T_PER_CORE = 100
SHARD = T_PER_CORE * P            # 12800
NSEG = 4
SEG_TILES = [31, 31, 28, 10]
SEG_T0 = [0, 31, 62, 90]
SEG_SLOTS = [st * P for st in SEG_TILES]           # per-core rows per seg
SEG_ROWS = [N_CORES * ss for ss in SEG_SLOTS]      # AG output rows (<32768)
G_TILES = 5
N_GROUPS = T_PER_CORE // G_TILES  # 20
IN_PAD = 640                      # x rows 0-511, b-row 512, zero pad
N_QUEUES = 4                      # SWDGE queues; q0 = casts, q1-3 = gathers
DEFER = [int(c) for c in os.environ.get("GCN_DEFER", "1256")]
DMAX = max(DEFER)
GBUFS = [min(max(DMAX - d + 2, 3), 7) for d in DEFER]
USE_FP8 = bool(int(os.environ.get("GCN_FP8", "1")))
SLACK0 = float(os.environ.get("GCN_SLACK", "0.05"))

BETAS = [math.log(THETA / (i + 1) + 1.0) for i in range(N_LAYERS)]


def _seg_of_tile(tl):
    for s in range(NSEG - 1, -1, -1):
        if tl >= SEG_T0[s]:
            return s
    return 0


# last group index whose completion finishes seg s (group g = tiles 5g..5g+4)
SEG_LAST_GROUP = [(SEG_T0[s] + SEG_TILES[s] - 1) // G_TILES for s in range(NSEG)]


# ----------------------------------------------------------------- host prep
def _wrap16(stream):
    n = stream.shape[0]
    wrap = stream.reshape(n // 16, 16).T.astype(np.int16)
    out = np.zeros((P, n // 16), dtype=np.int16)
    for rg in range(8):
        out[rg * 16:(rg + 1) * 16] = wrap
    return out


def _fit_caps(band_out_edges, slack):
    """caps[t][s]: chunk cap of cell (tile t, src band s), shared by cores."""
    caps = np.zeros((T_PER_CORE, NSEG), dtype=np.int64)
    for s in range(NSEG):
        per_core = band_out_edges[s] / N_CORES
        need = int(math.ceil(per_core * (1.0 + slack) / P))  # chunks per core
        base = need // T_PER_CORE
        rem = need - base * T_PER_CORE
        caps[:, s] = base
        if rem:
            # spread the +1 tiles evenly across tile indices
            idx = (np.arange(rem) * T_PER_CORE) // rem
            caps[idx, s] += 1
    return caps


def _assign_bands(cnt_out):
    """Band per node, degree-interleaved, sizes proportional to seg slots."""
    tgt = np.array([SEG_SLOTS[s] * N_CORES for s in range(NSEG)], dtype=np.float64)
    tgt = tgt / tgt.sum()
    order = np.argsort(-cnt_out, kind="stable")
    band_of = np.empty(N_NODES, dtype=np.int64)
    fills = np.zeros(NSEG)
    for i, v in enumerate(order):
        # deficit round-robin: pick band with largest (target*i - fill)
        s = int(np.argmax(tgt * (i + 1) - fills))
        band_of[v] = s
        fills[s] += 1
    return band_of


def _preprocess(x, edge_index):
    src = np.asarray(edge_index[0], dtype=np.int64)
    dst = np.asarray(edge_index[1], dtype=np.int64)

    deg = np.bincount(dst, minlength=N_NODES).astype(np.float64) + 1.0
    dinv = 1.0 / np.sqrt(deg)
    sqrtdeg = np.sqrt(deg)
    cnt_in = np.bincount(dst, minlength=N_NODES).astype(np.int64)
    cnt_out = np.bincount(src, minlength=N_NODES).astype(np.int64)

    band_of = _assign_bands(cnt_out)
    band_out_edges = np.array([cnt_out[band_of == s].sum() for s in range(NSEG)])

    n_tiles_total = N_CORES * T_PER_CORE
    order = np.argsort(-cnt_in, kind="stable")

    prof = np.zeros((N_NODES, NSEG), dtype=np.int64)
    np.add.at(prof, (dst, band_of[src]), 1)

    slack = SLACK0
    for attempt in range(4):
        caps = _fit_caps(band_out_edges, slack)
        caps_row = np.tile(caps * P, (N_CORES, 1))  # [800, NSEG] in rows

        tile_of = np.empty(N_NODES, dtype=np.int32)
        slot_of = np.empty(N_NODES, dtype=np.int32)
        tile_fill = np.zeros(n_tiles_total, dtype=np.int32)
        cell_load = np.zeros((n_tiles_total, NSEG), dtype=np.int64)
        ok_all = True
        for b in range(NSEG):
            tiles_b = np.array(
                [c * T_PER_CORE + SEG_T0[b] + j
                 for c in range(N_CORES) for j in range(SEG_TILES[b])])
            caps_b = caps_row[tiles_b].astype(np.float64)
            load_b = np.zeros((len(tiles_b), NSEG))
            fill_b = np.zeros(len(tiles_b))
            nodes_b = order[band_of[order] == b]
            assert len(nodes_b) <= len(tiles_b) * P, "band overflow"
            for v in nodes_b:
                pv = prof[v]
                newload = load_b + pv
                okm = (fill_b < P) & np.all(newload <= caps_b, axis=1)
                if not okm.any():
                    ok_all = False
                    break
                score = np.where(okm, (newload / np.maximum(caps_b, 1)).max(axis=1)
                                 + fill_b * 1e-4, np.inf)
                i = int(np.argmin(score))
                t = int(tiles_b[i])
                tile_of[v] = t
                slot_of[v] = int(fill_b[i])
                fill_b[i] += 1
                load_b[i] += pv
            if not ok_all:
                break
            tile_fill[tiles_b] = fill_b.astype(np.int32)
            cell_load[tiles_b] = load_b.astype(np.int64)
        if ok_all:
            break
        slack += 0.03
    assert ok_all, "packing failed at max slack"

    core_of = (tile_of // T_PER_CORE).astype(np.int32)
    tl = (tile_of % T_PER_CORE).astype(np.int64)
    seg_of = np.array([_seg_of_tile(t) for t in range(T_PER_CORE)])[tl]
    assert np.all(seg_of == band_of), "band/seg mismatch"

    # row within the seg's AG tensor — partition-major layout:
    # row = core*(P*ST) + slot*ST + (tile - seg_t0)
    st = np.array(SEG_TILES)[band_of]
    srow = (core_of.astype(np.int64) * (P * st)
            + slot_of * st + (tl - np.array(SEG_T0)[band_of]))

    e_src, e_dst = src, dst
    e_cell = tile_of[e_dst].astype(np.int64) * NSEG + band_of[e_src]
    cell_counts = np.bincount(e_cell, minlength=n_tiles_total * NSEG)
    assert np.all(cell_counts.reshape(N_CORES, T_PER_CORE, NSEG)
                  <= (caps * P)[None]), "cell overflow"
    order_e = np.argsort(e_cell, kind="stable")
    e_src, e_dst = e_src[order_e], e_dst[order_e]
    cell_starts = np.zeros(n_tiles_total * NSEG + 1, dtype=np.int64)
    np.cumsum(cell_counts, out=cell_starts[1:])

    x64 = np.asarray(x, dtype=np.float64)
    xT = (x64 * sqrtdeg[:, None]).T.astype(np.float16)

    per_core = []
    for c in range(N_CORES):
        gidx_cells = {}
        drel_cells = {}
        base = (c * T_PER_CORE) * NSEG
        for t in range(T_PER_CORE):
            for s in range(NSEG):
                cap = int(caps[t][s]) * P
                gi = np.zeros(cap, dtype=np.int16)
                dr = np.full(cap, -1.0, dtype=np.float16)
                cid = base + t * NSEG + s
                a, b = cell_starts[cid], cell_starts[cid + 1]
                n = b - a
                if n:
                    sub = np.argsort(srow[e_src[a:b]], kind="stable")
                    gi[:n] = srow[e_src[a:b]][sub].astype(np.int16)
                    dr[:n] = slot_of[e_dst[a:b]][sub].astype(np.float16)
                gidx_cells[(t, s)] = gi
                drel_cells[(t, s)] = dr

        blocks = []
        for g in range(N_GROUPS):
            for s in range(NSEG):
                stream = np.concatenate(
                    [gidx_cells[(g * G_TILES + i, s)] for i in range(G_TILES)])
                if stream.shape[0]:
                    blocks.append(_wrap16(stream))
        gidx_w = np.concatenate(blocks, axis=1)

        K_T = int(caps.sum(axis=1).max())
        drel_cols = np.zeros((P, T_PER_CORE * K_T), dtype=np.float16)
        for t in range(T_PER_CORE):
            c0 = t * K_T
            for s in range(NSEG):
                nch = int(caps[t][s])
                if nch:
                    drel_cols[:, c0:c0 + nch] = \
                        drel_cells[(t, s)].reshape(nch, P).T
                c0 += nch

        mask = core_of == c
        vids = np.nonzero(mask)[0]
        pos = tl[vids] * P + slot_of[vids]
        x_shard_T = np.zeros((IN_PAD, SHARD), dtype=np.float16)
        x_shard_T[:IN_DIM, pos] = xT[:, vids]
        x_shard_T[IN_DIM, pos] = sqrtdeg[vids].astype(np.float16)

        islot = slot_of[vids]
        itile = tl[vids]
        wscale = np.zeros((P, T_PER_CORE), dtype=np.float32)
        wscale[islot, itile] = (0.9 / deg[vids]).astype(np.float32)
        iscale = np.zeros((P, T_PER_CORE), dtype=np.float32)
        iscale[islot, itile] = (1.0 / deg[vids]).astype(np.float32)
        oscale = np.zeros(SHARD, dtype=np.float32)
        oscale[pos] = (0.9 * dinv[vids] / S_SCALE).astype(np.float32)

        per_core.append(dict(
            x_shard_T=x_shard_T,
            gidx=gidx_w,
            drel=np.ascontiguousarray(drel_cols),
            wscale=wscale,
            iscale=iscale,
            oscale=oscale,
        ))
    return per_core, core_of, tl, slot_of, caps


# ------------------------------------------------------------- device kernel
_BUILD_CACHE = {}


def _build(caps, n_layers=N_LAYERS):
    key = (tuple(map(tuple, caps)), n_layers, USE_FP8, tuple(DEFER))
    if key in _BUILD_CACHE:
        return _BUILD_CACHE[key]
    import concourse.bass as bass
    import concourse.bacc as bacc
    import concourse.tile as tile
    import concourse.mybir as mybir

    F32 = mybir.dt.float32
    F16 = mybir.dt.float16
    F8 = mybir.dt.float8e4
    I16 = mybir.dt.int16
    AT = mybir.AluOpType
    ts = bass.ts

    caps = np.asarray(caps)
    K_T = int(caps.sum(axis=1).max())
    # chunk base of (t, s) within tile t's drel column block
    CELL_BASE = [[int(caps[t, :s].sum()) for s in range(NSEG)]
                 for t in range(T_PER_CORE)]
    N_CH_T = [int(caps[t].sum()) for t in range(T_PER_CORE)]
    # chunks per gather call (g, s)
    CALL_CH = [[int(sum(caps[g * G_TILES + i, s] for i in range(G_TILES)))
                for s in range(NSEG)] for g in range(N_GROUPS)]
    TILE_OFF = [[[int(sum(caps[g * G_TILES + j, s] for j in range(i)))
                  for i in range(G_TILES)] for s in range(NSEG)]
                for g in range(N_GROUPS)]
    CALL_OFF = {}
    off = 0
    for g in range(N_GROUPS):
        for s in range(NSEG):
            CALL_OFF[(g, s)] = off
            off += CALL_CH[g][s] * P
    IDX_TOT = off
    MAXCC = [max(CALL_CH[g][s] for g in range(N_GROUPS)) for s in range(NSEG)]

    nc = bacc.Bacc("TRN2", target_bir_lowering=False, debug=False,
                   num_devices=N_CORES, num_swdge_queues=N_QUEUES)
    if N_QUEUES >= 4:
        # the 4th SWDGE queue only gets a DGE context slot if the unused
        # Activation HWDGE dynamic queue is not declared
        nc.hwdge_engines = type(nc.hwdge_engines)(
            [e for e in nc.hwdge_engines if e.name != "Activation"])
        nc.m.queues = [q for q in nc.m.queues if q.name != "qActDynamicHW"]

    x_in = nc.dram_tensor("x_shard_T", [IN_PAD, SHARD], F16, kind="ExternalInput")
    gidx_in = nc.dram_tensor("gidx", [P, IDX_TOT // 16], I16, kind="ExternalInput")
    drel_in = nc.dram_tensor("drel", [P, T_PER_CORE * K_T], F16, kind="ExternalInput")
    wscale_in = nc.dram_tensor("wscale", [P, T_PER_CORE], F32, kind="ExternalInput")
    iscale_in = nc.dram_tensor("iscale", [P, T_PER_CORE], F32, kind="ExternalInput")
    iota_in = nc.dram_tensor("iota_mod", [P, K_T * P], F16, kind="ExternalInput")
    ident_in = nc.dram_tensor("ident", [P, P], F16, kind="ExternalInput")
    win_in = nc.dram_tensor("W_in_stack", [P, IN_PAD], F16, kind="ExternalInput")
    wl_in = nc.dram_tensor("Wl_stack", [P, n_layers * HID], F16, kind="ExternalInput")
    wout_in = nc.dram_tensor("W_out_col", [P, 1], F16, kind="ExternalInput")

    out_t = nc.dram_tensor("out_shard", [1, SHARD], F32, kind="ExternalOutput")

    with tile.TileContext(nc) as tc:
        with (
            tc.tile_pool(name="res", bufs=1) as res,
            tc.tile_pool(name="gpool", bufs=2) as gpool,
            tc.tile_pool(name="spool", bufs=2) as spool,
            tc.tile_pool(name="work", bufs=3) as work,
            tc.tile_pool(name="ppool_a", bufs=2, space="PSUM") as ppool_a,
            tc.tile_pool(name="ppool_b", bufs=2, space="PSUM") as ppool_b,
            tc.tile_pool(name="ppool_c", bufs=2, space="PSUM") as ppool_c,
            tc.tile_pool(name="dram", bufs=1, space="DRAM") as dram,
        ):
            gidx_r = res.tile([P, IDX_TOT // 16], I16)
            drel_r = res.tile([P, T_PER_CORE * K_T], F16)
            wscale_r = res.tile([P, T_PER_CORE], F32)
            iscale_r = res.tile([P, T_PER_CORE], F32)
            iota_r = res.tile([P, K_T * P], F16)
            ident_r = res.tile([P, P], F16)
            win_r = res.tile([P, IN_PAD], F16)
            wl_r = res.tile([P, n_layers * HID], F16)
            wout_r = res.tile([P, 1], F16)
            x0s_r = res.tile([P, SHARD], F16)
            for sb, dr in [(gidx_r, gidx_in), (drel_r, drel_in),
                           (wscale_r, wscale_in), (iscale_r, iscale_in),
                           (iota_r, iota_in), (ident_r, ident_in),
                           (win_r, win_in), (wl_r, wl_in),
                           (wout_r, wout_in)]:
                nc.sync.dma_start(sb[:], dr[:])

            # xnext is partition-major: [p, t*HID+h] = node (t, slot p)
            xnext = dram.tile([P, T_PER_CORE * HID], F16)
            xf = [[dram.tile([SEG_ROWS[s], HID], F16,
                             addr_space="Local" if USE_FP8 else "Shared",
                             name=f"xf{i}_{s}")
                   for s in range(NSEG)] for i in range(n_layers)]
            xn8 = [dram.tile([P, SEG_TILES[s] * HID], F8 if USE_FP8 else F16,
                             name=f"xn8_{s}") for s in range(NSEG)]
            if USE_FP8:
                xf8 = [[dram.tile([SEG_ROWS[s], HID], F8, addr_space="Shared",
                                  name=f"xf8_{i}_{s}") for s in range(NSEG)]
                       for i in range(n_layers)]

            def write_xn8(g, stg):
                """ship group g's AG staging to the per-seg xn8 buffers."""
                i = 0
                while i < G_TILES:
                    t = g * G_TILES + i
                    s = _seg_of_tile(t)
                    j = i
                    while j < G_TILES and _seg_of_tile(g * G_TILES + j) == s:
                        j += 1
                    c0 = (g * G_TILES + i - SEG_T0[s]) * HID
                    nc.sync.dma_start(
                        xn8[s][:, c0:c0 + (j - i) * HID],
                        stg[:, i * P:j * P])
                    i = j

            def emit_ag(lidx, s):
                """AllGather for xf[lidx][s] from the per-seg staging."""
                nc.gpsimd.collective_compute(
                    "AllGather", mybir.AluOpType.bypass,
                    replica_groups=[list(range(N_CORES))],
                    ins=[xn8[s][:]],
                    outs=[(xf8 if USE_FP8 else xf)[lidx][s].opt()])

            def emit_upcast(lidx, s):
                """make gatherable f16 xf[lidx][s] from the AG output.

                Inner runs capped at 128 rows (16K elems) — multi-MB flat
                casts crash the SDMA conversion path.
                """
                if USE_FP8:
                    nc.gpsimd.dma_start(
                        xf[lidx][s][:].rearrange("(a b) h -> a b h", b=128),
                        xf8[lidx][s][:].rearrange("(a b) h -> a b h", b=128))

            # ---- initial projection
            init_scope = nc.enter_named_scope("init", False)
            for g in range(N_GROUPS):
                xng = work.tile([P, G_TILES * P], F16, name="xng", tag="xng",
                                bufs=2)
                xng8 = None
                if USE_FP8:
                    xng8 = work.tile([P, G_TILES * P], F8, name="xng8",
                                     tag="xng8", bufs=2)
                for i in range(G_TILES):
                    t = g * G_TILES + i
                    xt = work.tile([P, IN_PAD], F16, name="xt")
                    nc.sync.dma_start(
                        xt[:].rearrange("p (kb c) -> p kb c", kb=IN_PAD // P),
                        x_in[:, ts(t, P)]
                        .rearrange("(kb p) c -> p kb c", p=P))
                    ps_x = ppool_a.tile([P, P], F32, name="ps_x", tag="ps_agg")
                    for k in range(IN_PAD // P):
                        nc.tensor.matmul(
                            out=ps_x[:], lhsT=win_r[:, ts(k, P)],
                            rhs=xt[:, ts(k, P)],
                            start=(k == 0), stop=(k == IN_PAD // P - 1))
                    nc.vector.tensor_scalar(
                        out=x0s_r[:, ts(t, P)], in0=ps_x[:],
                        scalar1=ALPHA / 0.9, scalar2=None, op0=AT.mult)
                    ps_t = ppool_c.tile([P, P], F16, name="ps_t", tag="ps_t")
                    xps = work.tile([P, P], F16, name="xps")
                    nc.vector.tensor_copy(xps[:], ps_x[:])
                    nc.tensor.matmul(out=ps_t[:], lhsT=xps[:], rhs=ident_r[:],
                                     is_transpose=True)
                    nc.scalar.activation(
                        xng[:, ts(i, P)], ps_t[:],
                        mybir.ActivationFunctionType.Copy,
                        scale=iscale_r[:, t:t + 1])
                    if USE_FP8:
                        nc.scalar.activation(
                            xng8[:, ts(i, P)], ps_t[:],
                            mybir.ActivationFunctionType.Copy,
                            scale=iscale_r[:, t:t + 1])
                nc.sync.dma_start(xnext[:, ts(g, G_TILES * P)], xng[:])
                write_xn8(g, xng8 if USE_FP8 else xng)
                for s in range(NSEG):
                    if SEG_LAST_GROUP[s] == g:
                        emit_ag(0, s)
            nc.leave_named_scope("init", init_scope[0], False)

            # ---- layers
            qctr = 0
            for l in range(n_layers):
                lay_scope = nc.enter_named_scope(f"layer{l}", False)
                beta = BETAS[l]
                gbufs = {}          # (g, s) -> gbuf
                upcast_done = [False] * NSEG

                def gen_call(g, s, l=l):
                    nonlocal qctr
                    cc = CALL_CH[g][s]
                    if cc == 0:
                        gbufs[(g, s)] = None
                        return
                    blk = CALL_OFF[(g, s)] // 16
                    gbuf = gpool.tile([P, MAXCC[s] * P], F16, name=f"gbuf{s}",
                                      tag=f"gbuf{s}", bufs=GBUFS[s])
                    nc.gpsimd.dma_gather(
                        out_ap=gbuf[:, :cc * P].rearrange("p (c e) -> p c e", c=cc),
                        in_ap=xf[l][s][:],
                        idxs_ap=gidx_r[:, blk:blk + cc * P // 16],
                        num_idxs=cc * P, num_idxs_reg=cc * P, elem_size=HID,
                        single_packet=False,
                        queue_num=(0 if s == 3 else 1 + qctr % 3))
                    qctr += 1
                    gbufs[(g, s)] = gbuf

                def compute_group(g, l=l):
                    out_g = None
                    xng8 = None
                    if l == n_layers - 1:
                        out_g = work.tile([1, G_TILES * P], F32, name="out_g",
                                          tag="out_g", bufs=2)
                    elif USE_FP8:
                        xng8 = work.tile([P, G_TILES * P], F8, name="xng8",
                                         tag="xng8", bufs=2)
                    xself_g = work.tile([P, G_TILES * P], F16, name="xself_g",
                                        tag="xself", bufs=2)
                    nc.sync.dma_start(xself_g[:], xnext[:, ts(g, G_TILES * P)])
                    xng = work.tile([P, G_TILES * P], F16, name="xng",
                                    tag="xng", bufs=2)
                    for i in range(G_TILES):
                        t = g * G_TILES + i
                        nch = N_CH_T[t]
                        s_t = spool.tile([P, K_T * P], F16, name="s_t")
                        nc.vector.tensor_tensor(
                            out=s_t[:, :nch * P]
                            .rearrange("p (c e) -> p c e", c=nch),
                            in0=drel_r[:, t * K_T:t * K_T + nch]
                                .to_broadcast([P, nch, P]),
                            in1=iota_r[:, :nch * P]
                            .rearrange("p (c e) -> p c e", c=nch),
                            op=AT.is_equal)
                        ps_agg = ppool_a.tile([P, P], F32, name="ps_agg")
                        nc.tensor.matmul(out=ps_agg[:],
                                         lhsT=xself_g[:, ts(i, P)],
                                         rhs=ident_r[:], start=True, stop=False)
                        ch = 0
                        for s in range(NSEG):
                            off = TILE_OFF[g][s][i]
                            gb = gbufs[(g, s)]
                            for j in range(int(caps[t][s])):
                                nc.tensor.matmul(
                                    out=ps_agg[:],
                                    lhsT=gb[:, ts(off + j, P)],
                                    rhs=s_t[:, ts(ch, P)],
                                    start=False, stop=(ch == nch - 1))
                                ch += 1
                        h_t = work.tile([P, P], F16, name="h_t")
                        nc.vector.tensor_tensor(
                            out=h_t[:], in0=ps_agg[:], in1=x0s_r[:, ts(t, P)],
                            op=AT.add)
                        ps_d = ppool_b.tile([P, P], F32, name="ps_d")
                        nc.tensor.matmul(out=ps_d[:], lhsT=wl_r[:, ts(l, P)],
                                         rhs=h_t[:], start=True, stop=True)
                        xn_t = work.tile([P, P], F16, name="xn_t")
                        nc.scalar.activation(
                            xn_t[:], ps_d[:],
                            mybir.ActivationFunctionType.Relu, scale=1.0 - beta)
                        if l < n_layers - 1:
                            ps_t2 = ppool_c.tile([P, P], F16, name="ps_t2",
                                                 tag="ps_t")
                            nc.tensor.matmul(out=ps_t2[:], lhsT=xn_t[:],
                                             rhs=ident_r[:], is_transpose=True)
                            nc.scalar.activation(
                                xng[:, ts(i, P)], ps_t2[:],
                                mybir.ActivationFunctionType.Copy,
                                scale=wscale_r[:, t:t + 1])
                            if USE_FP8:
                                nc.scalar.activation(
                                    xng8[:, ts(i, P)], ps_t2[:],
                                    mybir.ActivationFunctionType.Copy,
                                    scale=wscale_r[:, t:t + 1])
                        else:
                            ps_o = ppool_b.tile([1, P], F32, name="ps_o",
                                                tag="ps_d")
                            nc.tensor.matmul(out=ps_o[:], lhsT=wout_r[:],
                                             rhs=xn_t[:], start=True, stop=True)
                            nc.scalar.copy(out_g[:, ts(i, P)], ps_o[:])
                    if l < n_layers - 1:
                        nc.sync.dma_start(xnext[:, ts(g, G_TILES * P)],
                                          xng[:])
                        write_xn8(g, xng8 if USE_FP8 else xng)
                    else:
                        nc.sync.dma_start(out_t[:, ts(g, G_TILES * P)],
                                          out_g[:])

                for step in range(N_GROUPS + DMAX):
                    for s in range(NSEG):
                        g = step - DEFER[s]
                        if 0 <= g < N_GROUPS:
                            if not upcast_done[s]:
                                emit_upcast(l, s)
                                upcast_done[s] = True
                            gen_call(g, s)
                    gc = step - DMAX
                    if 0 <= gc < N_GROUPS:
                        compute_group(gc)
                        if l < n_layers - 1:
                            for s in range(NSEG):
                                if SEG_LAST_GROUP[s] == gc:
                                    emit_ag(l + 1, s)
                nc.leave_named_scope(f"layer{l}", lay_scope[0], False)

    nc.compile()
    _BUILD_CACHE[key] = nc
    return nc


# ------------------------------------------------------------------ runner
def kernel(x, edge_index, edge_weight, W_in, b_in, W_layers, W_out, b_out):
    import concourse.bass_utils as bass_utils

    x = np.asarray(x)
    per_core, core_of, tl, slot_of, caps = _preprocess(x, edge_index)

    W_in = np.asarray(W_in, np.float32)
    b_in = np.asarray(b_in, np.float32)
    W_layers = np.asarray(W_layers, np.float32)
    W_out = np.asarray(W_out, np.float32)
    b_out = np.asarray(b_out, np.float32)

    K_T = int(caps.sum(axis=1).max())
    win_full = np.zeros((IN_PAD, HID), dtype=np.float32)
    win_full[:IN_DIM] = W_in * S_SCALE
    win_full[IN_DIM] = b_in * S_SCALE
    win_stack = win_full.reshape(IN_PAD // P, P, HID).transpose(1, 0, 2) \
                        .reshape(P, IN_PAD).astype(np.float16)
    eye = np.eye(HID, dtype=np.float64)
    wl_stack = np.concatenate(
        [eye + BETAS[l] / (1.0 - BETAS[l]) * W_layers[l].astype(np.float64)
         for l in range(N_LAYERS)],
        axis=1).astype(np.float16)
    iota_mod = np.broadcast_to(
        np.tile(np.arange(P, dtype=np.float32), K_T),
        (P, K_T * P)).astype(np.float16)
    ident = np.eye(P, dtype=np.float16)

    n_layers = int(os.environ.get('GCN_LAYERS', str(N_LAYERS)))
    in_maps = []
    for c in range(N_CORES):
        d = per_core[c]
        in_maps.append({
            "x_shard_T": d["x_shard_T"],
            "gidx": d["gidx"],
            "drel": d["drel"],
            "wscale": d["wscale"],
            "iscale": d["iscale"],
            "iota_mod": np.ascontiguousarray(iota_mod),
            "ident": ident,
            "W_in_stack": np.ascontiguousarray(win_stack),
            "Wl_stack": np.ascontiguousarray(wl_stack),
            "W_out_col": W_out.reshape(P, 1).astype(np.float16),
        })

    nc = _build(caps, n_layers)
    trace = bool(int(os.environ.get("GCN_TRACE", "0")))
    res = bass_utils.run_bass_kernel_spmd(
        nc, in_maps, core_ids=list(range(N_CORES)), trace=trace)
    kernel.last_results = res

    out = np.zeros((N_NODES, 1), dtype=np.float32)
    pos = tl * P + slot_of
    for c in range(N_CORES):
        mask = core_of == c
        raw = res.results[c]["out_shard"][0]
        osc = per_core[c]["oscale"]
        out[mask, 0] = raw[pos[mask]] * osc[pos[mask]] + b_out[0]
    return out
